# revision 1
# baseline (speedup 1.0000x reference)
"""L2 (spectral) contrastive loss on 8 Trainium2 NeuronCores.

Math: with G_x = x.T @ x and G_y = y.T @ y (both [D, D]),
    sum_{i,j} <x_i, y_j>^2 = ||x @ y.T||_F^2 = tr(G_x @ G_y) = sum(G_x * G_y)
so the loss needs only the two Gram matrices (2*N*D^2 MACs) instead of the
[N, N] pairwise product (N^2*D MACs).

Structure:
  - Rows of x and y are split across the 8 cores. Each core computes partial
    Grams over its 1024 rows in fp8e4 with the DoubleRow perf mode (two
    contraction rows per PE pass - 2x bf16 column throughput), fp32 PSUM
    accumulation, upper-triangle slabs only (Grams are symmetric).
  - The x2 weight for strict-upper-triangle entries AND a 2^-13 pre-scale
    (so fp16 dot products of ~8192-magnitude diag entries cannot overflow)
    are folded into the G_x pack copy (PSUM->SBUF on the scalar engine with
    scale), so the trace dot is three fused fp16 scalar_tensor_tensor ops
    with f32 column-sum accumulators, pipelined against the readback.
  - Diagonal terms z_i = <x_i, y_i> are computed from the fp32 inputs on
    the vector engine; their per-partition partials ride the packed
    AllReduce payload as two extra fp16 columns.
  - One fp16 AllReduce of [128, 5378] (both Gram triangles + z columns,
    ~1.38 MB) through the collectives firmware, then every core redundantly
    computes loss = (sum(Gx*Gy) - sum z^2)/(N*(N-1)) - (2/N)*sum z and
    core 0's output is returned.
  (A 3-round remote-DMA butterfly is ~40us faster than the collective but
  intermittently wedges the device - observed both in a prior session and
  reproduced here with the stock butterfly - so the collective stays.)
"""
import numpy as np
from contextlib import ExitStack

from concourse import bacc, tile, mybir
from concourse.bass_utils import run_bass_kernel_spmd

N_CORES = 8
N, D = 8192, 768
ROWS = N // N_CORES          # 1024 rows per core
P = 128                      # SBUF partitions
KCH = ROWS // P              # 8 contraction chunks per core
KK = KCH // 2                # 4 DoubleRow steps (2 chunks per pass)
MS = D // P                  # 6 output slabs per Gram

WIDTHS = [D - P * m for m in range(MS)]              # [768,640,512,384,256,128]
COFF = [sum(WIDTHS[:m]) for m in range(MS)]          # prefix offsets
GCOLS = sum(WIDTHS)                                  # 2688 per Gram
PACK_COLS = 2 * GCOLS + 2                            # + z columns
ZCOL = 2 * GCOLS

F32 = mybir.dt.float32
F16 = mybir.dt.float16
FP8 = mybir.dt.float8e4

# pack_x pre-scale: keeps the fp16 dot products (diag ~8192 * 8192) in
# fp16 range while keeping every pack_x entry comfortably normal
SCALE = 2.0 ** -13
HALF = GCOLS // 2

_CACHE = {}


def _mm_chunks(width):
    """Split [0, width) at the 512-column PSUM bank boundary."""
    if width <= 512:
        return [(0, width)]
    return [(0, 512), (512, width)]


def _build():
    nc = bacc.Bacc("TRN2", target_bir_lowering=False, debug=False,
                   num_devices=N_CORES)
    x_ap = nc.dram_tensor("x", [ROWS, D], F32, kind="ExternalInput").ap()
    y_ap = nc.dram_tensor("y", [ROWS, D], F32, kind="ExternalInput").ap()
    loss_ap = nc.dram_tensor("loss", [1, 1], F32, kind="ExternalOutput").ap()

    inv_nn1 = 1.0 / (float(N) * (N - 1))

    with tile.TileContext(nc) as tc:
        with ExitStack() as ctx:
            sb = ctx.enter_context(tc.tile_pool(name="sb", bufs=1))
            ps = ctx.enter_context(tc.tile_pool(name="ps", bufs=1, space="PSUM"))
            dram = ctx.enter_context(tc.tile_pool(name="dram", bufs=1,
                                                  space="DRAM"))

            # ---- load inputs: [1024, 768] -> [128p, 8k, 768], interleaved
            # so z and both Gram pipelines start on first arrival ----
            xt = sb.tile([P, KCH, D], F32)
            yt = sb.tile([P, KCH, D], F32)
            xr = x_ap.rearrange("(n p) d -> p n d", p=P)
            yr = y_ap.rearrange("(n p) d -> p n d", p=P)
            for k in range(KCH):
                nc.sync.dma_start(yt[:, k, :], yr[:, k, :])
                nc.sync.dma_start(xt[:, k, :], xr[:, k, :])

            # ---- cast to fp8e4 for the PE (y on gpsimd, x on scalar) ----
            xb = sb.tile([P, KCH, D], FP8)
            yb = sb.tile([P, KCH, D], FP8)
            for k in range(KCH):
                nc.gpsimd.tensor_copy(yb[:, k, :], yt[:, k, :])
                nc.scalar.copy(xb[:, k, :], xt[:, k, :])

            # ---- packed fp16 partials: [G_y | G_x(x2 off-diag) | z] ----
            pack = sb.tile([P, PACK_COLS], F16)

            # ---- diagonal terms z_i = <x_i, y_i> from fp32 (vector) ----
            # emitted BEFORE the Gram pack copies: the vector engine runs
            # these while the inputs stream in, not after the packs
            zscr = sb.tile([P, D], F32)
            zcols = sb.tile([P, KCH], F32)
            for k in range(KCH):
                nc.vector.scalar_tensor_tensor(
                    zscr[:], xt[:, k, :], 1.0, yt[:, k, :],
                    mybir.AluOpType.mult, mybir.AluOpType.mult,
                    accum_out=zcols[:, k:k + 1],
                )
            zsq = sb.tile([P, KCH], F32)
            zred = sb.tile([P, 2], F32)
            nc.vector.tensor_reduce(zred[:, 0:1], zcols[:],
                                    mybir.AxisListType.X, mybir.AluOpType.add)
            nc.vector.scalar_tensor_tensor(
                zsq[:], zcols[:], 1.0, zcols[:],
                mybir.AluOpType.mult, mybir.AluOpType.mult,
                accum_out=zred[:, 1:2],
            )
            # pre-scale so the fp16 z columns can't overflow after the
            # cross-core sum: col ZCOL = (2/N)*sum z, ZCOL+1 = inv_nn1*sum z^2
            zsc = sb.tile([P, 2], F32)
            nc.vector.tensor_scalar_mul(zsc[:, 0:1], zred[:, 0:1], 2.0 / N)
            nc.vector.tensor_scalar_mul(zsc[:, 1:2], zred[:, 1:2], inv_nn1)
            nc.vector.tensor_copy(pack[:, ZCOL:ZCOL + 2], zsc[:])
            ones = sb.tile([P, 1], F32)
            nc.vector.memset(ones[:], 1.0)

            cin = dram.tile([P, PACK_COLS], F16)
            cout = dram.tile([P, PACK_COLS], F16, addr_space="Shared")
            # ---- Grams: upper-triangle slabs, fp8 DoubleRow, fp32 PSUM ----
            for gi, src in enumerate((yb, xb)):
                for m in range(MS):
                    w = WIDTHS[m]
                    slab = ps.tile([P, w], F32, tag="slab", bufs=3,
                                   padded_shape=[P, 768])
                    for kk in range(KK):
                        for (c0, c1) in _mm_chunks(w):
                            nc.tensor.matmul(
                                slab[:, c0:c1],
                                src[:, 2 * kk:2 * kk + 2, P * m:P * (m + 1)],
                                src[:, 2 * kk:2 * kk + 2,
                                    P * m + c0:P * m + c1],
                                start=(kk == 0),
                                stop=(kk == KK - 1),
                                perf_mode=mybir.MatmulPerfMode.DoubleRow,
                                skip_group_check=True,
                            )
                    off = gi * GCOLS + COFF[m]
                    if gi == 0:   # G_y: plain copy (vector)
                        nc.vector.tensor_copy(pack[:, off:off + w],
                                              slab[:, 0:w])
                    else:         # G_x: pre-scaled, x2 on strict-right;
                        # copies alternate scalar/vector so the pack tail
                        # drains on two engines
                        if m % 2 == 0:
                            nc.scalar.mul(pack[:, off:off + P],
                                          slab[:, 0:P], SCALE)
                            if w > P:
                                nc.scalar.mul(pack[:, off + P:off + w],
                                              slab[:, P:w], 2.0 * SCALE)
                        else:
                            nc.vector.tensor_scalar_mul(
                                pack[:, off:off + P], slab[:, 0:P], SCALE)
                            if w > P:
                                nc.vector.tensor_scalar_mul(
                                    pack[:, off + P:off + w],
                                    slab[:, P:w], 2.0 * SCALE)
                        # stage this slab's cin columns immediately: the
                        # final staged piece is the 128-col slab, not a
                        # 344 KB half, pulling the collective trigger in
                        ceng = (nc.sync, nc.scalar, nc.gpsimd)[m % 3]
                        ceng.dma_start(cin[:, off:off + w],
                                       pack[:, off:off + w])

            # ---- single fp16 AllReduce of all partials ----
            # staged in pieces on separate queues: the G_y piece uploads
            # while the G_x Gram is still packing
            QW = GCOLS // 2
            nc.sync.dma_start(cin[:, 0:QW], pack[:, 0:QW])
            nc.gpsimd.dma_start(cin[:, QW:GCOLS], pack[:, QW:GCOLS])
            nc.gpsimd.dma_start(cin[:, ZCOL:PACK_COLS],
                                pack[:, ZCOL:PACK_COLS])
            nc.gpsimd.collective_compute(
                "AllReduce",
                mybir.AluOpType.add,
                replica_groups=[list(range(N_CORES))],
                ins=[cin.opt()],
                outs=[cout.opt()],
            )
            # ---- pipelined readback in column thirds across 3 DMA queues;
            # the dot runs as three fp16 STTs with f32 column accums, each
            # firing as soon as its (G_y, G_x) third has landed ----
            TH = GCOLS // 3
            gsum = sb.tile([P, PACK_COLS], F16)
            nc.sync.dma_start(gsum[:, 0:TH], cout[:, 0:TH])
            nc.scalar.dma_start(gsum[:, GCOLS:GCOLS + TH],
                                cout[:, GCOLS:GCOLS + TH])
            nc.gpsimd.dma_start(gsum[:, TH:2 * TH], cout[:, TH:2 * TH])
            nc.sync.dma_start(gsum[:, GCOLS + TH:GCOLS + 2 * TH],
                              cout[:, GCOLS + TH:GCOLS + 2 * TH])
            nc.scalar.dma_start(gsum[:, 2 * TH:GCOLS], cout[:, 2 * TH:GCOLS])
            nc.gpsimd.dma_start(gsum[:, GCOLS + 2 * TH:2 * GCOLS],
                                cout[:, GCOLS + 2 * TH:2 * GCOLS])
            nc.sync.dma_start(gsum[:, ZCOL:ZCOL + 2], cout[:, ZCOL:ZCOL + 2])

            stage = sb.tile([P, 6], F32)
            dscr = sb.tile([P, GCOLS], F16)
            for i in range(3):
                a, b = i * TH, (i + 1) * TH
                nc.vector.scalar_tensor_tensor(
                    dscr[:, a:b], gsum[:, GCOLS + a:GCOLS + b], 1.0,
                    gsum[:, a:b],
                    mybir.AluOpType.mult, mybir.AluOpType.mult,
                    accum_out=stage[:, i:i + 1],
                )
            nc.vector.tensor_copy(stage[:, 3:5], gsum[:, ZCOL:ZCOL + 2])
            nc.vector.memset(stage[:, 5:6], 0.0)

            # ---- partition reduction via PE (ones^T @ stage) ----
            pfin = ps.tile([1, 6], F32, tag="pfin", bufs=1)
            nc.tensor.matmul(pfin[0:1, 0:6], ones[:, 0:1], stage[:, 0:6],
                             start=True, stop=True)

            # loss = (d0+d1+d2)*inv_nn1/SCALE - zsq_scaled - zlin_scaled
            ffin = sb.tile([1, 6], F32)
            nc.vector.tensor_copy(ffin[:], pfin[0:1, 0:6])
            fres = sb.tile([1, 4], F32)
            nc.vector.scalar_tensor_tensor(
                fres[:, 0:1], ffin[:, 0:1], 1.0, ffin[:, 1:2],
                mybir.AluOpType.mult, mybir.AluOpType.add,
            )
            nc.vector.scalar_tensor_tensor(
                fres[:, 1:2], fres[:, 0:1], 1.0, ffin[:, 2:3],
                mybir.AluOpType.mult, mybir.AluOpType.add,
            )
            nc.vector.scalar_tensor_tensor(
                fres[:, 2:3], fres[:, 1:2], inv_nn1 / SCALE, ffin[:, 4:5],
                mybir.AluOpType.mult, mybir.AluOpType.subtract,
            )
            nc.vector.tensor_sub(fres[:, 3:4], fres[:, 2:3], ffin[:, 3:4])
            nc.sync.dma_start(loss_ap[:], fres[0:1, 3:4])

    nc.compile()
    return nc


def _get_nc():
    if "nc" not in _CACHE:
        _CACHE["nc"] = _build()
    return _CACHE["nc"]


def _run(x, y, trace=False, **trace_kwargs):
    nc = _get_nc()
    x = np.ascontiguousarray(np.asarray(x, dtype=np.float32))
    y = np.ascontiguousarray(np.asarray(y, dtype=np.float32))
    assert x.shape == (N, D) and y.shape == (N, D)
    in_maps = [
        {"x": x[c * ROWS:(c + 1) * ROWS], "y": y[c * ROWS:(c + 1) * ROWS]}
        for c in range(N_CORES)
    ]
    res = run_bass_kernel_spmd(nc, in_maps, list(range(N_CORES)), trace=trace,
                               **trace_kwargs)
    loss = np.float32(res.results[0]["loss"][0, 0])
    return np.asarray(loss, dtype=np.float32).reshape(()), res


def kernel(x, y):
    out, _ = _run(x, y, trace=False)
    return out

